# revision 38
# baseline (speedup 1.0000x reference)
"""Trainium2 Bass kernel for the AttentionLayer problem.

Math (per batch):
    Q = inp_q @ Wq + bq            [S, d]
    K = inp_k @ Wk + bk            [S, d]
    V = inp_v @ Wv + bv            [S, d]
    sc = Q @ K^T / sqrt(d)         [Sq, Sk]
    S_ = softmax(sc, axis=0)       (over the QUERY axis)
    H = S_ @ V                     [Sq, d]

Device-side layout strategy (per core, 2 batches):
  * Host feeds transposed activations xT = x^T [D, S] so every matmul
    contracts over the SBUF partition dim with zero on-chip transposes
    of the big activations.
  * Projections produce QT/KT/VT in [d, S] layout (d = 128 partitions).
  * scores^T [k, q] = (KT-slice)^T @ QT, so softmax-over-q is a
    free-axis row reduction: one ACT pass does exp(scale*x) and the
    row sum Z[k].  No max-subtraction is needed: |sc/sqrt(d)| <~ 6 for
    randn inputs, exp() is exact in f32 there.
  * Normalization is folded into V: vs[k, :] = V[k, :] / Z[k], then
    H^T [d, q] += vs-slice^T @ P^T accumulates over k-chunks in PSUM.
  * Host un-transposes H^T -> H.
Compute dtype bf16 (f32 PSUM accumulate), stats in f32.
"""

import math
import sys

sys.path.insert(0, "/opt/trn_rl_repo")

import ml_dtypes
import numpy as np

BF16_NP = ml_dtypes.bfloat16

import concourse.bass as bass  # noqa: E402
import concourse.tile as tile  # noqa: E402
from concourse import bacc, mybir  # noqa: E402

P = 128          # partitions / head dim d
S = 2048         # sequence length
D = 1024         # model dim
DC = D // P      # D chunks (8)
KC = S // P      # key chunks (16)
B_LOC = 2        # batches per core
N_CORES = 8
SCALE = 1.0 / math.sqrt(P)

F32 = mybir.dt.float32
BF16 = mybir.dt.bfloat16

_BUILT = None  # cached (nc,) so repeated kernel() calls reuse the NEFF


def build():
    nc = bacc.Bacc("TRN2", target_bir_lowering=False, debug=False,
                   num_devices=N_CORES)

    dr_in = {}
    for t in ("q", "k", "v"):
        dr_in[t] = nc.dram_tensor(f"{t}T", [B_LOC, D, S], BF16,
                                  kind="ExternalInput")
    # weights host-prepacked to the SBUF layout [p, c*P+e] so the load is
    # one contiguous 2KB run per partition on the HWDGE (sync) ring
    dr_w = {t: nc.dram_tensor(f"w{t}", [P, DC * P], BF16,
                              kind="ExternalInput")
            for t in ("q", "k", "v")}
    dr_b = {t: nc.dram_tensor(f"b{t}", [P], F32, kind="ExternalInput")
            for t in ("q", "k", "v")}
    dr_out = nc.dram_tensor("out", [B_LOC, P, S], BF16, kind="ExternalOutput")

    with tile.TileContext(nc) as tc:
        with (
            tc.tile_pool(name="const", bufs=1) as const,
            tc.tile_pool(name="stream", bufs=9) as stream,
            tc.tile_pool(name="proj", bufs=2) as proj,
            tc.tile_pool(name="kctp", bufs=10) as kctp,
            tc.tile_pool(name="ptp", bufs=18) as ptp,
            tc.tile_pool(name="stats", bufs=18) as stats,
            tc.tile_pool(name="recp", bufs=18) as recp,
            tc.tile_pool(name="zzp", bufs=18) as zzp,
            tc.tile_pool(name="osb", bufs=1) as osb,
            tc.tile_pool(name="ps_big", bufs=2, space="PSUM") as ps_big,
            tc.tile_pool(name="ps_acc", bufs=1, space="PSUM") as ps_acc,
        ):
            w_sb = {}
            b_sb = {}

            for t in ("q", "k", "v"):
                w_sb[t] = const.tile([P, DC, P], BF16, tag=f"w{t}",
                                     name=f"w{t}")
                b_sb[t] = const.tile([P, 1], F32, tag=f"b{t}", name=f"b{t}")
                nc.sync.dma_start(
                    b_sb[t][:],
                    dr_b[t].ap().rearrange("(p o) -> p o", o=1))

            # weight loads ride gpsimd (the sync/HWDGE ring streams bulk
            # data an order of magnitude slower here); host-packed layout
            # means one contiguous 2KB descriptor per partition.  Lazy
            # emission keeps w_k/w_v descriptor-gen off the Q7 until
            # after the early x-chunk DMAs are queued.
            _w_loaded = set()

            def ensure_w(t):
                if t in _w_loaded:
                    return
                _w_loaded.add(t)
                nc.gpsimd.dma_start(
                    w_sb[t][:],
                    dr_w[t].ap().rearrange("p (c e) -> p c e", e=P))

            def load_x_dbl(t, b, cc, eng):
                x = stream.tile([P, 2, S], BF16, tag="stream", name="x")
                eng.dma_start(
                    x[:],
                    dr_in[t].ap()[b, cc * 2 * P:(cc + 1) * 2 * P, :]
                    .rearrange("(two p) s -> p two s", two=2))
                return x
            # V bias as a rank-1 matmul (ones[1,128].T @ bias_row[1,128])
            # appended to each V accumulation group; created lazily so
            # these ops don't delay the first q-chunk DMA on gpsimd
            _vbias_box = []

            def ensure_vbias():
                if not _vbias_box:
                    ones_row = const.tile([1, P], BF16, tag="ones",
                                          name="ones_row")
                    nc.vector.memset(ones_row[:], 1.0)
                    bv_row = const.tile([1, P], BF16, tag="bvr",
                                        name="bv_row")
                    nc.gpsimd.dma_start(
                        bv_row[:],
                        dr_b["v"].ap().rearrange("(o e) -> o e", o=1))
                    _vbias_box.append((ones_row, bv_row))
                return _vbias_box[0]

            def proj_dbl_chunk(t, b, cc, sinks):
                """Load a double D-chunk (two 128-row slabs in one
                dma_start for better DMA efficiency) and run its
                projection matmuls.  sinks(c, rhs_slice_fn) emits them."""
                ensure_w(t)
                x = load_x_dbl(t, b, cc, nc.gpsimd)
                for two in range(2):
                    sinks(cc * 2 + two, x[:, two, :])

            def emit_qt(b, t="q", tag="qT"):
                """Q/K projection: 4 double-chunks -> [d, S] bf16.
                Batch 1's accumulator lives in the "acc" PSUM rotation
                (one 4-bank tile) so its projection can run during
                batch 0's exp-paced scores chain instead of waiting for
                batch 0's sc double-buffer to drain."""
                if b == 0:
                    halves = [ps_big.tile([P, 1024], F32, tag="big",
                                          name="q_ps") for _ in range(2)]
                else:
                    qh = ps_acc.tile([P, 2, 1024], F32, tag="acc",
                                     name="qh1")
                    halves = [qh[:, h, :] for h in range(2)]

                def sinks(c, rhs):
                    for h in range(2):
                        for s2 in range(2):
                            nc.tensor.matmul(
                                halves[h][:, s2 * 512:(s2 + 1) * 512],
                                lhsT=w_sb[t][:, c, :],
                                rhs=rhs[:, h * 1024 + s2 * 512:
                                        h * 1024 + (s2 + 1) * 512],
                                start=(c == 0), stop=(c == DC - 1))

                for cc in range(DC // 2):
                    proj_dbl_chunk(t, b, cc, sinks)
                out = proj.tile([P, S], BF16, tag=tag, name=tag)
                for h in range(2):
                    nc.vector.tensor_scalar_add(
                        out[:, h * 1024:(h + 1) * 1024],
                        halves[h][:], b_sb[t][:])
                return out

            def emit_vnat_chunk(b, v_ps, cc):
                """One double D-chunk of the V projection, computed
                directly in natural [S, d] layout: the input slab slices
                are the stationary operands, so no PE transpose or
                extra SBUF staging is needed afterwards."""

                def sinks(c, rhs):
                    # start=True clears the WHOLE psum bank, and four
                    # [128,128] V regions share each bank — so only the
                    # first region per bank issues the clearing start;
                    # the rest overwrite-on-first-write via the cleared
                    # has_written bits.
                    for sc in range(KC):
                        nc.tensor.matmul(
                            v_ps[:, sc, :],
                            lhsT=rhs[:, sc * P:(sc + 1) * P],
                            rhs=w_sb["v"][:, c, :],
                            start=(c == 0 and sc % 4 == 0),
                            stop=False)

                proj_dbl_chunk("v", b, cc, sinks)

            def emit_v_finish(v_ps):
                """Rank-1 bias add (ones^T @ bias_row) closes each
                accumulation group, then copy V to SBUF bf16."""
                ones_row, bv_row = ensure_vbias()
                for sc in range(KC):
                    nc.tensor.matmul(
                        v_ps[:, sc, :], lhsT=ones_row[:], rhs=bv_row[:],
                        start=False, stop=True)
                v_sb = proj.tile([P, KC, P], BF16, tag="v", name="v")
                for g in range(2):
                    nc.vector.tensor_copy(
                        v_sb[:, g * 8:(g + 1) * 8, :],
                        v_ps[:, g * 8:(g + 1) * 8, :])
                return v_sb

            def emit_kslab(b, sl):
                """K super-chunk: one [D, 256] slab -> kct [d, 256] bf16
                (2 k-chunks worth of KT), so scores start on the first
                slab instead of after the whole K projection.  The slab
                accumulator lives in the "acc" PSUM rotation, which is
                idle during the scores chain — the sc double-buffer in
                "big" stays undisturbed."""
                ensure_w("k")
                xk = stream.tile([P, DC, 256], BF16, tag="stream",
                                 name="xk")
                nc.gpsimd.dma_start(
                    xk[:],
                    dr_in["k"].ap()[b, :, sl * 256:(sl + 1) * 256]
                    .rearrange("(c p) s -> p c s", p=P))
                kps = ps_acc.tile([P, 256], F32, tag="acc", name="kps")
                for c in range(DC):
                    nc.tensor.matmul(
                        kps[:], lhsT=w_sb["k"][:, c, :], rhs=xk[:, c, :],
                        start=(c == 0), stop=(c == DC - 1))
                kct = kctp.tile([P, 256], BF16, tag="kt", name="kct")
                nc.vector.tensor_scalar_add(kct[:], kps[:], b_sb["k"][:])
                return kct

            def emit_scores(qt, lhsT_ap):
                """One k-chunk of scores^T + exp + Z accumulate."""
                pt = ptp.tile([P, S], BF16, tag="pt", name="pt")
                zz = zzp.tile([P, 2], F32, tag="z", name="zz")
                for h in range(2):
                    sc = ps_big.tile([P, 1024], F32, tag="big",
                                     name="sc_ps")
                    for s2 in range(2):
                        nc.tensor.matmul(
                            sc[:, s2 * 512:(s2 + 1) * 512],
                            lhsT=lhsT_ap,
                            rhs=qt[:, h * 1024 + s2 * 512:
                                   h * 1024 + (s2 + 1) * 512],
                            start=True, stop=True)
                    nc.scalar.activation(
                        pt[:, h * 1024:(h + 1) * 1024], sc[:],
                        func=mybir.ActivationFunctionType.Exp,
                        scale=SCALE, accum_out=zz[:, h:h + 1])
                return pt, zz

            def emit_h_and_out(b, v_sb, pts, recs, last):
                """H accumulation kc-outer (all 4 q-slices per k-chunk)
                so only the last k-chunk's 4 matmuls trail the final
                exp.  For the final batch the tail casts split across
                DVE and ACT so the two engines drain the last ht banks
                in parallel; earlier batches keep ACT free for the next
                batch's exps."""
                ht = ps_acc.tile([P, S], F32, tag="acc", name="ht")
                out_sb = osb.tile([P, S], BF16, tag="osb", name="out_sb")
                for kc in range(KC):
                    vs = stats.tile([P, P], BF16, tag="vs", name="vs")
                    nc.vector.tensor_scalar_mul(vs[:], v_sb[:, kc, :],
                                                recs[kc][:])
                    for st in range(4):
                        nc.tensor.matmul(
                            ht[:, st * 512:(st + 1) * 512],
                            lhsT=vs[:],
                            rhs=pts[kc][:, st * 512:(st + 1) * 512],
                            start=(kc == 0), stop=(kc == KC - 1))
                for st in range(4):
                    sl = slice(st * 512, (st + 1) * 512)
                    if st % 2 == 0 or not last:
                        nc.vector.tensor_copy(out_sb[:, sl], ht[:, sl])
                    else:
                        nc.scalar.activation(
                            out_sb[:, sl], ht[:, sl],
                            func=mybir.ActivationFunctionType.Copy)
                    nc.sync.dma_start(dr_out.ap()[b][:, sl],
                                      out_sb[:, sl])

            def emit_rec(zz):
                rec = recp.tile([P, 1], F32, tag="rec", name="rec")
                nc.vector.tensor_reduce(
                    rec[:], zz[:], axis=mybir.AxisListType.X,
                    op=mybir.AluOpType.add)
                nc.vector.reciprocal(rec[:], rec[:])
                return rec

            # K in [D, 256] slabs fused with the scores/exp chain; each
            # slab's projection is emitted `ahead` slabs before its
            # scores so the kps-matmul + kct-copy latency hides under
            # earlier slabs' exp ops.  1/Z rides the DVE stream at
            # lag-8 behind its exp (dependency long satisfied) so no
            # later kct copy ever waits on an in-flight exp.
            def chain_step(b, qt, kcts, pts, zzs, recs, sl, ahead):
                nxt = sl + ahead
                if nxt < KC // 2:
                    kcts.append(emit_kslab(b, nxt))
                for j in range(2):
                    pt, zz = emit_scores(
                        qt, kcts[sl][:, j * P:(j + 1) * P])
                    pts.append(pt)
                    zzs.append(zz)
                    kc = 2 * sl + j
                    if kc >= 8:
                        recs.append(emit_rec(zzs[kc - 8]))

            def emit_back(b, pts, zzs, recs, last):
                """Remaining recs + V projection + H + output store."""
                for kc in range(len(recs), KC):
                    recs.append(emit_rec(zzs[kc]))
                v_ps = ps_acc.tile([P, KC, P], F32, tag="acc",
                                   name="v_ps")
                for cc in range(DC // 2):
                    emit_vnat_chunk(b, v_ps, cc)
                v_sb = emit_v_finish(v_ps)
                emit_h_and_out(b, v_sb, pts, recs, last)

            # ---- batch 0 front + scores chain through slab 6 ----
            qt0 = emit_qt(0)
            k0 = [emit_kslab(0, 0)]
            p0, z0, r0 = [], [], []
            for sl in range(7):
                chain_step(0, qt0, k0, p0, z0, r0, sl, 1)

            # ---- hoist batch 1's front end: its Q projection (into
            # the spare "acc" PSUM rotation) and first 3 K slabs run on
            # the PE during batch 0's exp-paced chain, so batch 1's exp
            # chain starts the moment batch 0's last exp retires
            # instead of after batch 0's V/H/out ----
            qt1 = emit_qt(1)
            k1 = [emit_kslab(1, i) for i in range(3)]

            # ---- batch 0: last slab (kc14/15) ----
            chain_step(0, qt0, k0, p0, z0, r0, 7, 1)

            # ---- batch 1's first two chain steps (kc0-3) are also
            # hoisted ahead of batch 0's back end: the PE runs in
            # program order, so these score matmuls must sit before
            # batch 0's V/H block or batch 1's exps can't start until
            # it drains.  Their sc slots free as batch 0's last exps
            # retire, so the ACT chain rolls straight from batch 0
            # into batch 1 ----
            p1, z1, r1 = [], [], []
            for sl in range(2):
                chain_step(1, qt1, k1, p1, z1, r1, sl, 3)

            # ---- batch 0 back end (casts DVE-only) ----
            emit_back(0, p0, z0, r0, last=False)

            # ---- batch 1: rest of the scores chain + back end ----
            for sl in range(2, 8):
                chain_step(1, qt1, k1, p1, z1, r1, sl, 3)
            emit_back(1, p1, z1, r1, last=True)

    nc.compile()
    return nc


def _get_nc():
    global _BUILT
    if _BUILT is None:
        _BUILT = build()
    return _BUILT


def pack_w(wk):
    """[D, P] f32 -> [P, DC*P] bf16 in the on-chip [p, c, e] layout."""
    wk = np.asarray(wk, dtype=np.float32)
    return np.ascontiguousarray(
        wk.reshape(DC, P, P).transpose(1, 0, 2).reshape(P, DC * P)
    ).astype(BF16_NP)


def kernel(inp_q, inp_k, inp_v, Wq_kernel, Wq_bias, Wk_kernel, Wk_bias,
           Wv_kernel, Wv_bias):
    from concourse.bass_utils import run_bass_kernel_spmd

    nc = _get_nc()

    inp = {"q": np.asarray(inp_q, dtype=np.float32),
           "k": np.asarray(inp_k, dtype=np.float32),
           "v": np.asarray(inp_v, dtype=np.float32)}
    # pack [D, P] -> [P, DC*P] bf16: row p holds W[c*128+p, :] for c=0..7
    w = {"q": pack_w(Wq_kernel), "k": pack_w(Wk_kernel),
         "v": pack_w(Wv_kernel)}
    bias = {"q": np.ascontiguousarray(np.asarray(Wq_bias, dtype=np.float32)),
            "k": np.ascontiguousarray(np.asarray(Wk_bias, dtype=np.float32)),
            "v": np.ascontiguousarray(np.asarray(Wv_bias, dtype=np.float32))}

    in_maps = []
    for c in range(N_CORES):
        m = {}
        for t in ("q", "k", "v"):
            # [2, S, D] -> [2, D, S] bf16 contiguous (layout + dtype
            # marshalling on host; halves device HBM traffic)
            m[f"{t}T"] = inp[t][c * B_LOC:(c + 1) * B_LOC] \
                .transpose(0, 2, 1).astype(BF16_NP)
            m[f"w{t}"] = w[t]
            m[f"b{t}"] = bias[t]
        in_maps.append(m)

    res = run_bass_kernel_spmd(nc, in_maps, list(range(N_CORES)))

    out = np.empty((N_CORES * B_LOC, S, P), dtype=np.float32)
    for c in range(N_CORES):
        # [2, P, S] bf16 -> [2, S, P] f32
        out[c * B_LOC:(c + 1) * B_LOC] = (
            res.results[c]["out"].astype(np.float32).transpose(0, 2, 1))
    return out

